# revision 2
# baseline (speedup 1.0000x reference)
"""CLDice loss Trainium2 kernel (v2).

Sharding: 8 cores = (batch b, z-half, y-half) quarters with 12-voxel halos on
interior z/y cut sides; lower/right shards are z/y-flipped by the host so the
true volume border is always at index 0.

Per core the soft-skeletonization runs entirely in SBUF:
  - pred path: bf16 [z=108 partitions, (y=108)x(x=192) free]; 3^3 min/max
    pools decompose per axis into 2 tensor_tensor ops; z +-1 shifts are
    SBUF->SBUF partition-shifted DMAs (no PE matmuls / PSUM);
    the x stage pairs taps as s'[x]=op(d[x-1],d[x+1]) then op(s',d) so only
    one op is 4B-misaligned.
  - skeleton update uses skel += delta*(1-skel) with delta = e_j - open(e_j)
    which is >= 0 exactly (opening <= id holds exactly for min/max), so no
    relu ops are needed.
  - gt path: bit-packed uint32 boolean morphology (AND/OR), same structure.
Partial sums are combined on the host into the scalar loss.
"""
import numpy as np

NCORES = 8
Z = Y = X = 192
ZO = YO = 96          # owned extent per quarter
HALO = 12             # 11 erodes + 1 dilate
ZL = YL = ZO + HALO   # local extended extent (108)
NW = 8                # words per row in packed gt (cols 1..6 data, 0/7 pads)
ND = 6                # data words per row
NIT = 11              # skeletonize iterations (incl. k=0 init)
FCH = 24              # final-pass row chunk
FY = YL * X
ONES = 0xFFFFFFFF

_CACHE = {}

import ml_dtypes as _mld


def _build(reps=1):
    import concourse.bacc as bacc
    import concourse.bass as bass
    import concourse.mybir as mybir
    from concourse import tile
    from concourse.alu_op_type import AluOpType as aop
    from contextlib import nullcontext

    dt = mybir.dt
    AF = mybir.ActivationFunctionType
    nc = bacc.Bacc("TRN2", target_bir_lowering=False, debug=False,
                   num_devices=NCORES)

    lg_d = nc.dram_tensor("lg", [2, ZL, YL, X], dt.bfloat16, kind="ExternalInput").ap()
    gtb_d = nc.dram_tensor("gtb", [ZL, YL, NW], dt.uint32, kind="ExternalInput").ap()
    p0_d = nc.dram_tensor("p0", [ZO, YO * X], dt.bfloat16).ap()
    sums_d = nc.dram_tensor("sums", [128, 16], dt.float32, kind="ExternalOutput").ap()

    with tile.TileContext(nc) as tc:
        with tc.tile_pool(name="perm", bufs=1) as perm:
            skel = perm.tile([ZO, YO * X], dt.bfloat16)
            skg = perm.tile([ZO, YO * NW], dt.uint32)
            acc = perm.tile([128, 16], dt.float32)
            loop = tc.For_i(0, reps, 1) if reps > 1 else nullcontext()
            with loop:
                nc.vector.memset(acc[:, :], 0.0)
                with tc.tile_pool(name="mp", bufs=1) as mp:
                    Ea = mp.tile([ZL, FY], dt.bfloat16, name="Ea")
                    Eb = mp.tile([ZL, FY], dt.bfloat16, name="Eb")
                    S1 = mp.tile([ZL, FY], dt.bfloat16, name="S1")
                    D1 = mp.tile([ZL, 51 * X], dt.bfloat16, name="D1")
                    DTt = mp.tile([ZO, 12 * X], dt.bfloat16, name="DTt")
                    Ga = mp.tile([ZL, YL * NW], dt.uint32, name="Ga")
                    Gb = mp.tile([ZL, YL * NW], dt.uint32, name="Gb")
                    T1 = mp.tile([ZL, YL * NW], dt.uint32, name="T1")
                    T2 = mp.tile([ZL, YL * NW], dt.uint32, name="T2")
                    T3 = mp.tile([ZL, YL * NW], dt.uint32, name="T3")

                    # ---------------- init: sigmoid(l1-l0) -> Ea --------------
                    nc.vector.memset(T1[:, :], 0)
                    nc.vector.memset(T2[:, :], 0)
                    nc.vector.memset(T3[:, :], 0)
                    nc.sync.dma_start(S1[:, :], lg_d[0, :, :, :])
                    nc.sync.dma_start(Eb[:, :], lg_d[1, :, :, :])
                    nc.sync.dma_start(Ga[:, :], gtb_d[:, :, :])
                    nc.vector.tensor_sub(S1[:, :], Eb[:, :], S1[:, :])
                    nc.scalar.activation(Ea[:, :], S1[:, :], AF.Sigmoid)
                    nc.sync.dma_start(p0_d[:, :], Ea[0:ZO, 0:YO * X])

                    # ---------------- float erode: A -> B (via S1) ------------
                    def erode(A, B):
                        R = YL - 1  # 107
                        # z stage: b = min(A, A[z+1], A[z-1]); the up shift
                        # lands in B (free early) so only the dn shift waits
                        # for S1 to drain from the previous dilate.
                        nc.sync.dma_start(B[0:ZL - 1, :], A[1:ZL, :])
                        nc.scalar.dma_start(S1[1:ZL, :], A[0:ZL - 1, :])
                        nc.scalar.dma_start(S1[0:1, :], A[0:1, :])
                        nc.vector.tensor_tensor(B[:, :], A[:, :], B[:, :], aop.min)
                        nc.vector.tensor_tensor(B[:, :], B[:, :], S1[:, :], aop.min)
                        # y stage: c[y]=min(b[y],b[y+1]); d[0]=c[0]; d[y]=min(c[y-1],c[y])
                        nc.vector.tensor_tensor(S1[:, 0:R * X], B[:, 0:R * X], B[:, X:YL * X], aop.min)
                        nc.vector.tensor_copy(B[:, 0:X], S1[:, 0:X])
                        nc.vector.tensor_tensor(B[:, X:R * X], S1[:, 0:(R - 1) * X], S1[:, X:R * X], aop.min)
                        # x stage on rows 0..R-1
                        B3 = B.rearrange("p (r c) -> p r c", c=X)
                        S3 = S1.rearrange("p (r c) -> p r c", c=X)
                        nc.vector.tensor_tensor(S3[:, 0:R, 0:X - 2], B3[:, 0:R, 0:X - 2], B3[:, 0:R, 2:X], aop.min)
                        nc.vector.tensor_tensor(B3[:, 0:R, X - 1:X], B3[:, 0:R, X - 2:X - 1], B3[:, 0:R, X - 1:X], aop.min)
                        nc.vector.tensor_tensor(B3[:, 0:R, 0:1], B3[:, 0:R, 0:1], B3[:, 0:R, 1:2], aop.min)
                        nc.vector.tensor_tensor(B3[:, 0:R, 1:X - 1], S3[:, 0:R, 0:X - 2], B3[:, 0:R, 1:X - 1], aop.min)

                    # -------- float dilate of B (=e_{j+1}) + delta/skel -------
                    def dilate_delta(A, B, j):
                        ZP = ZO + 1  # 97: z rows 0..96 of the dilate
                        for y0, y1 in ((0, 48), (48, YO)):
                            w0 = max(y0 - 1, 0)
                            w1 = y1 + 1
                            L = w1 - w0           # 49 / 50
                            Lc = y1 - w0          # 48 / 49
                            LX = L * X
                            # z stage on y rows w0..w1-1
                            nc.sync.dma_start(S1[0:ZP, 0:LX], B[1:ZP + 1, w0 * X:w1 * X])
                            nc.vector.tensor_tensor(S1[0:ZP, 0:LX], B[0:ZP, w0 * X:w1 * X], S1[0:ZP, 0:LX], aop.max)
                            nc.scalar.dma_start(D1[1:ZP, 0:LX], B[0:ZP - 1, w0 * X:w1 * X])
                            nc.scalar.dma_start(D1[0:1, 0:LX], B[0:1, w0 * X:w1 * X])
                            nc.vector.tensor_tensor(D1[0:ZP, 0:LX], S1[0:ZP, 0:LX], D1[0:ZP, 0:LX], aop.max)
                            # y stage
                            nc.vector.tensor_tensor(S1[0:ZP, 0:Lc * X], D1[0:ZP, 0:Lc * X], D1[0:ZP, X:(Lc + 1) * X], aop.max)
                            if y0 == 0:
                                nc.vector.tensor_copy(D1[0:ZP, 0:X], S1[0:ZP, 0:X])
                                nc.vector.tensor_tensor(D1[0:ZP, X:48 * X], S1[0:ZP, 0:47 * X], S1[0:ZP, X:48 * X], aop.max)
                            else:
                                nc.vector.tensor_tensor(D1[0:ZP, 0:48 * X], S1[0:ZP, 0:48 * X], S1[0:ZP, X:49 * X], aop.max)
                            # x stage rows 0..47
                            RD = 48
                            D3 = D1.rearrange("p (r c) -> p r c", c=X)
                            S3 = S1.rearrange("p (r c) -> p r c", c=X)
                            nc.vector.tensor_tensor(S3[0:ZP, 0:RD, 0:X - 2], D3[0:ZP, 0:RD, 0:X - 2], D3[0:ZP, 0:RD, 2:X], aop.max)
                            nc.vector.tensor_tensor(D3[0:ZP, 0:RD, X - 1:X], D3[0:ZP, 0:RD, X - 2:X - 1], D3[0:ZP, 0:RD, X - 1:X], aop.max)
                            nc.vector.tensor_tensor(D3[0:ZP, 0:RD, 0:1], D3[0:ZP, 0:RD, 0:1], D3[0:ZP, 0:RD, 1:2], aop.max)
                            nc.vector.tensor_tensor(D3[0:ZP, 0:RD, 1:X - 1], S3[0:ZP, 0:RD, 0:X - 2], D3[0:ZP, 0:RD, 1:X - 1], aop.max)
                            # delta/skel on 12-row chunks of the owned half
                            for q in range(4):
                                g0 = y0 + q * 12
                                l0 = q * 12
                                cs = slice(g0 * X, (g0 + 12) * X)
                                ls = slice(l0 * X, (l0 + 12) * X)
                                nc.gpsimd.tensor_sub(DTt[:, :], A[0:ZO, cs], D1[0:ZO, ls])
                                if j == 0:
                                    nc.vector.tensor_copy(skel[:, cs], DTt[:, :])
                                else:
                                    nc.scalar.activation(D1[0:ZO, ls], skel[:, cs], AF.Copy, scale=-1.0, bias=1.0)
                                    nc.gpsimd.tensor_mul(DTt[:, :], DTt[:, :], D1[0:ZO, ls])
                                    nc.gpsimd.tensor_add(skel[:, cs], skel[:, cs], DTt[:, :])

                    # ---------------- packed-gt helpers -----------------------
                    def gt_pads(G, val):
                        G3 = G.rearrange("p (r w) -> p r w", w=NW)
                        nc.vector.memset(G3[:, :, 0:1], val)
                        nc.vector.memset(G3[:, :, 7:8], val)

                    def gt_erode(G, H):
                        R = YL - 1
                        gt_pads(G, ONES)
                        nc.sync.dma_start(T1[0:ZL - 1, :], G[1:ZL, :])
                        nc.vector.tensor_tensor(T1[:, :], G[:, :], T1[:, :], aop.bitwise_and)
                        nc.sync.dma_start(H[1:ZL, :], G[0:ZL - 1, :])
                        nc.vector.memset(H[0:1, :], ONES)
                        nc.vector.tensor_tensor(H[:, :], T1[:, :], H[:, :], aop.bitwise_and)
                        nc.vector.tensor_tensor(T1[:, 0:R * NW], H[:, 0:R * NW], H[:, NW:YL * NW], aop.bitwise_and)
                        nc.vector.tensor_copy(H[:, 0:NW], T1[:, 0:NW])
                        nc.vector.tensor_tensor(H[:, NW:R * NW], T1[:, 0:(R - 1) * NW], T1[:, NW:R * NW], aop.bitwise_and)
                        # x bits: out = (H & t_minus) & t_plus
                        H3 = H.rearrange("p (r w) -> p r w", w=NW)
                        t13 = T1.rearrange("p (r w) -> p r w", w=NW)
                        t23 = T2.rearrange("p (r w) -> p r w", w=NW)
                        t33 = T3.rearrange("p (r w) -> p r w", w=NW)
                        v = slice(0, R)
                        nc.vector.tensor_single_scalar(t13[:, v, 1:7], H3[:, v, 1:7], 1, aop.logical_shift_left)
                        nc.vector.tensor_single_scalar(t23[:, v, 1:7], H3[:, v, 0:6], 31, aop.logical_shift_right)
                        nc.vector.tensor_tensor(t13[:, v, 1:7], t13[:, v, 1:7], t23[:, v, 1:7], aop.bitwise_or)
                        nc.vector.tensor_tensor(t13[:, v, 1:7], t13[:, v, 1:7], H3[:, v, 1:7], aop.bitwise_and)
                        nc.vector.tensor_single_scalar(t23[:, v, 1:7], H3[:, v, 1:7], 1, aop.logical_shift_right)
                        nc.vector.tensor_single_scalar(t33[:, v, 1:7], H3[:, v, 2:8], 31, aop.logical_shift_left)
                        nc.vector.tensor_tensor(t23[:, v, 1:7], t23[:, v, 1:7], t33[:, v, 1:7], aop.bitwise_or)
                        nc.vector.tensor_tensor(H3[:, v, 1:7], t13[:, v, 1:7], t23[:, v, 1:7], aop.bitwise_and)

                    def gt_dilate_skg(G, H, j):
                        R = YL - 1
                        gt_pads(H, 0)
                        nc.sync.dma_start(T1[0:ZL - 1, :], H[1:ZL, :])
                        nc.vector.tensor_tensor(T1[:, :], H[:, :], T1[:, :], aop.bitwise_or)
                        nc.sync.dma_start(T2[1:ZL, :], H[0:ZL - 1, :])
                        nc.vector.memset(T2[0:1, :], 0)
                        nc.vector.tensor_tensor(T2[:, :], T1[:, :], T2[:, :], aop.bitwise_or)
                        nc.vector.tensor_tensor(T1[:, 0:R * NW], T2[:, 0:R * NW], T2[:, NW:YL * NW], aop.bitwise_or)
                        nc.vector.tensor_copy(T2[:, 0:NW], T1[:, 0:NW])
                        nc.vector.tensor_tensor(T2[:, NW:R * NW], T1[:, 0:(R - 1) * NW], T1[:, NW:R * NW], aop.bitwise_or)
                        # x bits (OR distributes): GD = d | d<<1 | prev>>31 | d>>1 | next<<31
                        t13 = T1.rearrange("p (r w) -> p r w", w=NW)
                        t23 = T2.rearrange("p (r w) -> p r w", w=NW)
                        t33 = T3.rearrange("p (r w) -> p r w", w=NW)
                        v = slice(0, R)
                        nc.vector.tensor_single_scalar(t13[:, v, 1:7], t23[:, v, 1:7], 1, aop.logical_shift_left)
                        nc.vector.tensor_tensor(t13[:, v, 1:7], t13[:, v, 1:7], t23[:, v, 1:7], aop.bitwise_or)
                        nc.vector.tensor_single_scalar(t33[:, v, 1:7], t23[:, v, 0:6], 31, aop.logical_shift_right)
                        nc.vector.tensor_tensor(t13[:, v, 1:7], t13[:, v, 1:7], t33[:, v, 1:7], aop.bitwise_or)
                        nc.vector.tensor_single_scalar(t33[:, v, 1:7], t23[:, v, 1:7], 1, aop.logical_shift_right)
                        nc.vector.tensor_tensor(t13[:, v, 1:7], t13[:, v, 1:7], t33[:, v, 1:7], aop.bitwise_or)
                        nc.vector.tensor_single_scalar(t33[:, v, 1:7], t23[:, v, 2:8], 31, aop.logical_shift_left)
                        nc.vector.tensor_tensor(t13[:, v, 1:7], t13[:, v, 1:7], t33[:, v, 1:7], aop.bitwise_or)
                        # gnt = G & ~GD on owned; fold into skg
                        G3 = G.rearrange("p (r w) -> p r w", w=NW)
                        skg3 = skg.rearrange("p (r w) -> p r w", w=NW)
                        nc.vector.tensor_single_scalar(t13[0:ZO, 0:YO, 1:7], t13[0:ZO, 0:YO, 1:7], ONES, aop.bitwise_xor)
                        nc.vector.tensor_tensor(t13[0:ZO, 0:YO, 1:7], G3[0:ZO, 0:YO, 1:7], t13[0:ZO, 0:YO, 1:7], aop.bitwise_and)
                        if j == 0:
                            nc.vector.tensor_copy(skg3[:, :, 1:7], t13[0:ZO, 0:YO, 1:7])
                        else:
                            nc.vector.tensor_tensor(skg3[:, :, 1:7], skg3[:, :, 1:7], t13[0:ZO, 0:YO, 1:7], aop.bitwise_or)

                    # ---------------- main iterations -------------------------
                    cur, nxt = Ea, Eb
                    gcur, gnxt = Ga, Gb
                    for j in range(NIT):
                        erode(cur, nxt)
                        dilate_delta(cur, nxt, j)
                        gt_erode(gcur, gnxt)
                        gt_dilate_skg(gcur, gnxt, j)
                        cur, nxt = nxt, cur
                        gcur, gnxt = gnxt, gcur

                # ---------------- final: partial sums ----------------------
                with tc.tile_pool(name="fin", bufs=1) as fp:
                    for ci in range(4):
                        y0, y1 = ci * FCH, (ci + 1) * FCH
                        rr = y1 - y0
                        FR = rr * X
                        gtw = fp.tile([ZO, FCH * NW], dt.uint32, tag="fgw")
                        nc.sync.dma_start(gtw[:, :], gtb_d[0:ZO, y0:y1, :])
                        mi = fp.tile([ZO, FCH * X], dt.uint32, tag="fmi")
                        mi4 = mi.rearrange("p (r w b) -> p r w b", w=ND, b=32)
                        gw4 = gtw.rearrange("p (r w) -> p r w", w=NW)
                        for b in range(32):
                            nc.vector.tensor_scalar(
                                mi4[:, 0:rr, :, b], gw4[:, 0:rr, 1:7], b, 1,
                                aop.logical_shift_right, aop.bitwise_and)
                        mb = fp.tile([ZO, FCH * X], dt.bfloat16, tag="fmb")
                        nc.scalar.copy(mb[:, :FR], mi[:, :FR])
                        scr = fp.tile([ZO, FCH * X], dt.bfloat16, tag="fsc")
                        # S1 = sum(skel_pred * gt)
                        nc.vector.scalar_tensor_tensor(
                            scr[:, :FR], skel[:, y0 * X:y1 * X], 1.0, mb[:, :FR],
                            aop.mult, aop.mult, accum_out=acc[0:ZO, ci:ci + 1])
                        # S2 = sum(skel_pred)
                        nc.vector.tensor_scalar(
                            scr[:, :FR], skel[:, y0 * X:y1 * X], 0.0, 0.0,
                            aop.add, aop.add, accum_out=acc[0:ZO, 4 + ci:5 + ci])
                        # unpack skel_gt
                        sg4 = skg.rearrange("p (r w) -> p r w", w=NW)
                        for b in range(32):
                            nc.vector.tensor_scalar(
                                mi4[:, 0:rr, :, b], sg4[:, y0:y1, 1:7], b, 1,
                                aop.logical_shift_right, aop.bitwise_and)
                        nc.scalar.copy(mb[:, :FR], mi[:, :FR])
                        # S4 = sum(skel_gt)
                        nc.vector.tensor_scalar(
                            scr[:, :FR], mb[:, :FR], 0.0, 0.0,
                            aop.add, aop.add, accum_out=acc[0:ZO, 12 + ci:13 + ci])
                        # S3 = sum(skel_gt * pred)
                        pt = fp.tile([ZO, FCH * X], dt.bfloat16, tag="fpt")
                        nc.sync.dma_start(pt[:, :FR], p0_d[:, y0 * X:y1 * X])
                        nc.vector.scalar_tensor_tensor(
                            scr[:, :FR], mb[:, :FR], 1.0, pt[:, :FR],
                            aop.mult, aop.mult, accum_out=acc[0:ZO, 8 + ci:9 + ci])
                nc.sync.dma_start(sums_d[:, :], acc[:, :])

    nc.compile()
    return nc


def _host_shard(logits, targets):
    logits = np.ascontiguousarray(np.asarray(logits, dtype=np.float32))
    targets = np.asarray(targets)
    in_maps = []
    for c in range(NCORES):
        b, zh, yh = c >> 2, (c >> 1) & 1, c & 1
        lg = logits[b]
        gt = (targets[b] == 1)
        if zh:
            lg = lg[:, ::-1]
            gt = gt[::-1]
        if yh:
            lg = lg[:, :, ::-1]
            gt = gt[:, ::-1]
        lg = np.ascontiguousarray(lg[:, :ZL, :YL]).astype(_mld.bfloat16)
        gt = np.ascontiguousarray(gt[:ZL, :YL])               # (ZL, YL, X) bool
        words = np.packbits(gt, axis=-1, bitorder="little")   # (ZL, YL, 24) u8
        words = words.view(np.uint32)                         # (ZL, YL, 6)
        gtb = np.zeros((ZL, YL, NW), dtype=np.uint32)
        gtb[:, :, 1:7] = words
        in_maps.append({"lg": lg, "gtb": gtb})
    return in_maps


def _reduce_sums(per_core_sums):
    S = np.zeros(4, dtype=np.float64)
    for a in per_core_sums:
        a = a.astype(np.float64)
        S[0] += a[:, 0:4].sum()
        S[1] += a[:, 4:8].sum()
        S[2] += a[:, 8:12].sum()
        S[3] += a[:, 12:16].sum()
    tprec = (S[0] + 1.0) / (S[1] + 1.0)
    tsens = (S[2] + 1.0) / (S[3] + 1.0)
    cl = 2.0 * tprec * tsens / (tprec + tsens + 1e-7)
    return np.float32(1.0 - cl)


def kernel(logits, targets):
    from concourse.bass_utils import run_bass_kernel_spmd
    if "nc" not in _CACHE:
        _CACHE["nc"] = _build()
    nc = _CACHE["nc"]
    in_maps = _host_shard(logits, targets)
    res = run_bass_kernel_spmd(nc, in_maps, list(range(NCORES)), trace=False)
    return _reduce_sums([r["sums"] for r in res.results])


# revision 3
# speedup vs baseline: 1.9285x; 1.9285x over previous
"""CLDice loss Trainium2 kernel (v2).

Sharding: 8 cores = (batch b, z-half, y-half) quarters with 12-voxel halos on
interior z/y cut sides; lower/right shards are z/y-flipped by the host so the
true volume border is always at index 0.

Per core the soft-skeletonization runs entirely in SBUF:
  - pred path: bf16 [z=108 partitions, (y=108)x(x=192) free]; 3^3 min/max
    pools decompose per axis into 2 tensor_tensor ops; z +-1 shifts are
    SBUF->SBUF partition-shifted DMAs (no PE matmuls / PSUM);
    the x stage pairs taps as s'[x]=op(d[x-1],d[x+1]) then op(s',d) so only
    one op is 4B-misaligned.
  - skeleton update uses skel += delta*(1-skel) with delta = e_j - open(e_j)
    which is >= 0 exactly (opening <= id holds exactly for min/max), so no
    relu ops are needed.
  - gt path: bit-packed uint32 boolean morphology (AND/OR), same structure.
Partial sums are combined on the host into the scalar loss.
"""
import numpy as np

NCORES = 8
Z = Y = X = 192
ZO = YO = 96          # owned extent per quarter
HALO = 12             # 11 erodes + 1 dilate
ZL = YL = ZO + HALO   # local extended extent (108)
NW = 8                # words per row in packed gt (cols 1..6 data, 0/7 pads)
ND = 6                # data words per row
NIT = 11              # skeletonize iterations (incl. k=0 init)
FCH = 24              # final-pass row chunk
FY = YL * X
ONES = 0xFFFFFFFF

_CACHE = {}

import ml_dtypes as _mld


def _build(reps=1):
    import concourse.bacc as bacc
    import concourse.bass as bass
    import concourse.mybir as mybir
    from concourse import tile
    from concourse.alu_op_type import AluOpType as aop
    from contextlib import nullcontext

    dt = mybir.dt
    AF = mybir.ActivationFunctionType
    nc = bacc.Bacc("TRN2", target_bir_lowering=False, debug=False,
                   num_devices=NCORES)

    lg_d = nc.dram_tensor("lg", [2, ZL, YL, X], dt.bfloat16, kind="ExternalInput").ap()
    gtb_d = nc.dram_tensor("gtb", [ZL, YL, NW], dt.uint32, kind="ExternalInput").ap()
    gtf_d = nc.dram_tensor("gtf", [ZO, YO * X], dt.bfloat16, kind="ExternalInput").ap()
    p0_d = nc.dram_tensor("p0", [ZO, YO * X], dt.bfloat16).ap()
    sums_d = nc.dram_tensor("sums", [128, 16], dt.float32, kind="ExternalOutput").ap()

    with tile.TileContext(nc) as tc:
        with tc.tile_pool(name="perm", bufs=1) as perm:
            skel = perm.tile([ZO, YO * X], dt.bfloat16)
            skg = perm.tile([ZO, YO * NW], dt.uint32)
            acc = perm.tile([128, 16], dt.float32)
            loop = tc.For_i(0, reps, 1) if reps > 1 else nullcontext()
            with loop:
                nc.vector.memset(acc[:, :], 0.0)
                with tc.tile_pool(name="mp", bufs=1) as mp:
                    Ea = mp.tile([ZL, FY], dt.bfloat16, name="Ea")
                    Eb = mp.tile([ZL, FY], dt.bfloat16, name="Eb")
                    S1 = mp.tile([ZL, FY], dt.bfloat16, name="S1")
                    D1 = mp.tile([ZL, 51 * X], dt.bfloat16, name="D1")
                    DTt = mp.tile([ZO, 16 * X], dt.bfloat16, name="DTt")
                    Ga = mp.tile([ZL, YL * NW], dt.uint32, name="Ga")
                    Gb = mp.tile([ZL, YL * NW], dt.uint32, name="Gb")
                    T1 = mp.tile([ZL, YL * NW], dt.uint32, name="T1")
                    T2 = mp.tile([ZL, YL * NW], dt.uint32, name="T2")
                    T3 = mp.tile([ZL, YL * NW], dt.uint32, name="T3")

                    # ---------------- init: sigmoid(l1-l0) -> Ea --------------
                    nc.vector.memset(T1[:, :], 0)
                    nc.vector.memset(T2[:, :], 0)
                    nc.vector.memset(T3[:, :], 0)
                    nc.sync.dma_start(S1[:, :], lg_d[0, :, :, :])
                    nc.sync.dma_start(Eb[:, :], lg_d[1, :, :, :])
                    nc.sync.dma_start(Ga[:, :], gtb_d[:, :, :])
                    nc.vector.tensor_sub(S1[:, :], Eb[:, :], S1[:, :])
                    nc.scalar.activation(Ea[:, :], S1[:, :], AF.Sigmoid)
                    nc.sync.dma_start(p0_d[:, :], Ea[0:ZO, 0:YO * X])

                    # ---------------- float erode: A -> B (via S1) ------------
                    def erode(A, B):
                        R = YL - 1  # 107
                        # z stage: b = min(A, A[z+1], A[z-1]); the up shift
                        # lands in B (free early) so only the dn shift waits
                        # for S1 to drain from the previous dilate.
                        nc.sync.dma_start(B[0:ZL - 1, :], A[1:ZL, :])
                        nc.scalar.dma_start(S1[1:ZL, :], A[0:ZL - 1, :])
                        nc.scalar.dma_start(S1[0:1, :], A[0:1, :])
                        nc.vector.tensor_tensor(B[:, :], A[:, :], B[:, :], aop.min)
                        nc.vector.tensor_tensor(B[:, :], B[:, :], S1[:, :], aop.min)
                        # y stage: c[y]=min(b[y],b[y+1]); d[0]=c[0]; d[y]=min(c[y-1],c[y])
                        nc.vector.tensor_tensor(S1[:, 0:R * X], B[:, 0:R * X], B[:, X:YL * X], aop.min)
                        nc.scalar.copy(B[:, 0:X], S1[:, 0:X])
                        nc.vector.tensor_tensor(B[:, X:R * X], S1[:, 0:(R - 1) * X], S1[:, X:R * X], aop.min)
                        # x stage on rows 0..R-1
                        B3 = B.rearrange("p (r c) -> p r c", c=X)
                        S3 = S1.rearrange("p (r c) -> p r c", c=X)
                        nc.vector.tensor_tensor(S3[:, 0:R, 0:X - 2], B3[:, 0:R, 0:X - 2], B3[:, 0:R, 2:X], aop.min)
                        nc.vector.tensor_tensor(B3[:, 0:R, X - 1:X], B3[:, 0:R, X - 2:X - 1], B3[:, 0:R, X - 1:X], aop.min)
                        nc.vector.tensor_tensor(B3[:, 0:R, 0:1], B3[:, 0:R, 0:1], B3[:, 0:R, 1:2], aop.min)
                        nc.vector.tensor_tensor(B3[:, 0:R, 1:X - 1], S3[:, 0:R, 0:X - 2], B3[:, 0:R, 1:X - 1], aop.min)

                    # -------- float dilate of B (=e_{j+1}) + delta/skel -------
                    def dilate_delta(A, B, j):
                        ZP = ZO + 1  # 97: z rows 0..96 of the dilate
                        for y0, y1 in ((0, 48), (48, YO)):
                            w0 = max(y0 - 1, 0)
                            w1 = y1 + 1
                            L = w1 - w0           # 49 / 50
                            Lc = y1 - w0          # 48 / 49
                            LX = L * X
                            # z stage on y rows w0..w1-1
                            nc.sync.dma_start(S1[0:ZP, 0:LX], B[1:ZP + 1, w0 * X:w1 * X])
                            nc.vector.tensor_tensor(S1[0:ZP, 0:LX], B[0:ZP, w0 * X:w1 * X], S1[0:ZP, 0:LX], aop.max)
                            nc.scalar.dma_start(D1[1:ZP, 0:LX], B[0:ZP - 1, w0 * X:w1 * X])
                            nc.scalar.dma_start(D1[0:1, 0:LX], B[0:1, w0 * X:w1 * X])
                            nc.vector.tensor_tensor(D1[0:ZP, 0:LX], S1[0:ZP, 0:LX], D1[0:ZP, 0:LX], aop.max)
                            # y stage
                            nc.vector.tensor_tensor(S1[0:ZP, 0:Lc * X], D1[0:ZP, 0:Lc * X], D1[0:ZP, X:(Lc + 1) * X], aop.max)
                            if y0 == 0:
                                nc.scalar.copy(D1[0:ZP, 0:X], S1[0:ZP, 0:X])
                                nc.vector.tensor_tensor(D1[0:ZP, X:48 * X], S1[0:ZP, 0:47 * X], S1[0:ZP, X:48 * X], aop.max)
                            else:
                                nc.vector.tensor_tensor(D1[0:ZP, 0:48 * X], S1[0:ZP, 0:48 * X], S1[0:ZP, X:49 * X], aop.max)
                            # x stage rows 0..47
                            RD = 48
                            D3 = D1.rearrange("p (r c) -> p r c", c=X)
                            S3 = S1.rearrange("p (r c) -> p r c", c=X)
                            nc.vector.tensor_tensor(S3[0:ZP, 0:RD, 0:X - 2], D3[0:ZP, 0:RD, 0:X - 2], D3[0:ZP, 0:RD, 2:X], aop.max)
                            nc.vector.tensor_tensor(D3[0:ZP, 0:RD, X - 1:X], D3[0:ZP, 0:RD, X - 2:X - 1], D3[0:ZP, 0:RD, X - 1:X], aop.max)
                            nc.vector.tensor_tensor(D3[0:ZP, 0:RD, 0:1], D3[0:ZP, 0:RD, 0:1], D3[0:ZP, 0:RD, 1:2], aop.max)
                            nc.vector.tensor_tensor(D3[0:ZP, 0:RD, 1:X - 1], S3[0:ZP, 0:RD, 0:X - 2], D3[0:ZP, 0:RD, 1:X - 1], aop.max)
                            # delta/skel on 16-row chunks of the owned half
                            for q in range(3):
                                g0 = y0 + q * 16
                                l0 = q * 16
                                cs = slice(g0 * X, (g0 + 16) * X)
                                ls = slice(l0 * X, (l0 + 16) * X)
                                nc.gpsimd.tensor_sub(DTt[:, :], A[0:ZO, cs], D1[0:ZO, ls])
                                if j == 0:
                                    nc.vector.tensor_copy(skel[:, cs], DTt[:, :])
                                else:
                                    nc.scalar.activation(D1[0:ZO, ls], skel[:, cs], AF.Copy, scale=-1.0, bias=1.0)
                                    nc.gpsimd.tensor_mul(DTt[:, :], DTt[:, :], D1[0:ZO, ls])
                                    nc.gpsimd.tensor_add(skel[:, cs], skel[:, cs], DTt[:, :])

                    # ---------------- packed-gt helpers -----------------------
                    def gt_pads(G, val):
                        G3 = G.rearrange("p (r w) -> p r w", w=NW)
                        nc.vector.memset(G3[:, :, 0:1], val)
                        nc.vector.memset(G3[:, :, 7:8], val)

                    def gt_erode(G, H):
                        R = YL - 1
                        gt_pads(G, ONES)
                        nc.sync.dma_start(T1[0:ZL - 1, :], G[1:ZL, :])
                        nc.vector.tensor_tensor(T1[:, :], G[:, :], T1[:, :], aop.bitwise_and)
                        nc.sync.dma_start(H[1:ZL, :], G[0:ZL - 1, :])
                        nc.vector.memset(H[0:1, :], ONES)
                        nc.vector.tensor_tensor(H[:, :], T1[:, :], H[:, :], aop.bitwise_and)
                        nc.vector.tensor_tensor(T1[:, 0:R * NW], H[:, 0:R * NW], H[:, NW:YL * NW], aop.bitwise_and)
                        nc.vector.tensor_copy(H[:, 0:NW], T1[:, 0:NW])
                        nc.vector.tensor_tensor(H[:, NW:R * NW], T1[:, 0:(R - 1) * NW], T1[:, NW:R * NW], aop.bitwise_and)
                        # x bits: out = (H & t_minus) & t_plus
                        H3 = H.rearrange("p (r w) -> p r w", w=NW)
                        t13 = T1.rearrange("p (r w) -> p r w", w=NW)
                        t23 = T2.rearrange("p (r w) -> p r w", w=NW)
                        t33 = T3.rearrange("p (r w) -> p r w", w=NW)
                        v = slice(0, R)
                        nc.vector.tensor_single_scalar(t13[:, v, 1:7], H3[:, v, 1:7], 1, aop.logical_shift_left)
                        nc.vector.tensor_single_scalar(t23[:, v, 1:7], H3[:, v, 0:6], 31, aop.logical_shift_right)
                        nc.vector.tensor_tensor(t13[:, v, 1:7], t13[:, v, 1:7], t23[:, v, 1:7], aop.bitwise_or)
                        nc.vector.tensor_tensor(t13[:, v, 1:7], t13[:, v, 1:7], H3[:, v, 1:7], aop.bitwise_and)
                        nc.vector.tensor_single_scalar(t23[:, v, 1:7], H3[:, v, 1:7], 1, aop.logical_shift_right)
                        nc.vector.tensor_single_scalar(t33[:, v, 1:7], H3[:, v, 2:8], 31, aop.logical_shift_left)
                        nc.vector.tensor_tensor(t23[:, v, 1:7], t23[:, v, 1:7], t33[:, v, 1:7], aop.bitwise_or)
                        nc.vector.tensor_tensor(H3[:, v, 1:7], t13[:, v, 1:7], t23[:, v, 1:7], aop.bitwise_and)

                    def gt_dilate_skg(G, H, j):
                        R = YL - 1
                        gt_pads(H, 0)
                        nc.sync.dma_start(T1[0:ZL - 1, :], H[1:ZL, :])
                        nc.vector.tensor_tensor(T1[:, :], H[:, :], T1[:, :], aop.bitwise_or)
                        nc.sync.dma_start(T2[1:ZL, :], H[0:ZL - 1, :])
                        nc.vector.memset(T2[0:1, :], 0)
                        nc.vector.tensor_tensor(T2[:, :], T1[:, :], T2[:, :], aop.bitwise_or)
                        nc.vector.tensor_tensor(T1[:, 0:R * NW], T2[:, 0:R * NW], T2[:, NW:YL * NW], aop.bitwise_or)
                        nc.vector.tensor_copy(T2[:, 0:NW], T1[:, 0:NW])
                        nc.vector.tensor_tensor(T2[:, NW:R * NW], T1[:, 0:(R - 1) * NW], T1[:, NW:R * NW], aop.bitwise_or)
                        # x bits (OR distributes): GD = d | d<<1 | prev>>31 | d>>1 | next<<31
                        t13 = T1.rearrange("p (r w) -> p r w", w=NW)
                        t23 = T2.rearrange("p (r w) -> p r w", w=NW)
                        t33 = T3.rearrange("p (r w) -> p r w", w=NW)
                        v = slice(0, R)
                        nc.vector.tensor_single_scalar(t13[:, v, 1:7], t23[:, v, 1:7], 1, aop.logical_shift_left)
                        nc.vector.tensor_tensor(t13[:, v, 1:7], t13[:, v, 1:7], t23[:, v, 1:7], aop.bitwise_or)
                        nc.vector.tensor_single_scalar(t33[:, v, 1:7], t23[:, v, 0:6], 31, aop.logical_shift_right)
                        nc.vector.tensor_tensor(t13[:, v, 1:7], t13[:, v, 1:7], t33[:, v, 1:7], aop.bitwise_or)
                        nc.vector.tensor_single_scalar(t33[:, v, 1:7], t23[:, v, 1:7], 1, aop.logical_shift_right)
                        nc.vector.tensor_tensor(t13[:, v, 1:7], t13[:, v, 1:7], t33[:, v, 1:7], aop.bitwise_or)
                        nc.vector.tensor_single_scalar(t33[:, v, 1:7], t23[:, v, 2:8], 31, aop.logical_shift_left)
                        nc.vector.tensor_tensor(t13[:, v, 1:7], t13[:, v, 1:7], t33[:, v, 1:7], aop.bitwise_or)
                        # gnt = G & ~GD on owned; fold into skg
                        G3 = G.rearrange("p (r w) -> p r w", w=NW)
                        skg3 = skg.rearrange("p (r w) -> p r w", w=NW)
                        nc.vector.tensor_single_scalar(t13[0:ZO, 0:YO, 1:7], t13[0:ZO, 0:YO, 1:7], ONES, aop.bitwise_xor)
                        nc.vector.tensor_tensor(t13[0:ZO, 0:YO, 1:7], G3[0:ZO, 0:YO, 1:7], t13[0:ZO, 0:YO, 1:7], aop.bitwise_and)
                        if j == 0:
                            nc.vector.tensor_copy(skg3[:, :, 1:7], t13[0:ZO, 0:YO, 1:7])
                        else:
                            nc.vector.tensor_tensor(skg3[:, :, 1:7], skg3[:, :, 1:7], t13[0:ZO, 0:YO, 1:7], aop.bitwise_or)

                    # ---------------- main iterations -------------------------
                    cur, nxt = Ea, Eb
                    gcur, gnxt = Ga, Gb
                    for j in range(NIT):
                        erode(cur, nxt)
                        dilate_delta(cur, nxt, j)
                        gt_erode(gcur, gnxt)
                        gt_dilate_skg(gcur, gnxt, j)
                        cur, nxt = nxt, cur
                        gcur, gnxt = gnxt, gcur

                # ---------------- final: partial sums ----------------------
                with tc.tile_pool(name="fin", bufs=1) as fp:
                    for ci in range(4):
                        y0, y1 = ci * FCH, (ci + 1) * FCH
                        rr = y1 - y0
                        FR = rr * X
                        # S1 = sum(skel_pred * gt): gt comes in as bf16
                        gf = fp.tile([ZO, FCH * X], dt.bfloat16, tag="fgf")
                        nc.sync.dma_start(gf[:, :FR], gtf_d[:, y0 * X:y1 * X])
                        scr = fp.tile([ZO, FCH * X], dt.bfloat16, tag="fsc")
                        nc.vector.scalar_tensor_tensor(
                            scr[:, :FR], skel[:, y0 * X:y1 * X], 1.0, gf[:, :FR],
                            aop.mult, aop.mult, accum_out=acc[0:ZO, ci:ci + 1])
                        # S2 = sum(skel_pred)
                        nc.vector.tensor_scalar(
                            scr[:, :FR], skel[:, y0 * X:y1 * X], 0.0, 0.0,
                            aop.add, aop.add, accum_out=acc[0:ZO, 4 + ci:5 + ci])
                        # unpack skel_gt
                        mi = fp.tile([ZO, FCH * X], dt.uint32, tag="fmi")
                        mi4 = mi.rearrange("p (r w b) -> p r w b", w=ND, b=32)
                        sg4 = skg.rearrange("p (r w) -> p r w", w=NW)
                        for b in range(32):
                            nc.vector.tensor_scalar(
                                mi4[:, 0:rr, :, b], sg4[:, y0:y1, 1:7], b, 1,
                                aop.logical_shift_right, aop.bitwise_and)
                        mb = fp.tile([ZO, FCH * X], dt.bfloat16, tag="fmb")
                        nc.scalar.copy(mb[:, :FR], mi[:, :FR])
                        # S4 = sum(skel_gt)
                        nc.vector.tensor_scalar(
                            scr[:, :FR], mb[:, :FR], 0.0, 0.0,
                            aop.add, aop.add, accum_out=acc[0:ZO, 12 + ci:13 + ci])
                        # S3 = sum(skel_gt * pred)
                        pt = fp.tile([ZO, FCH * X], dt.bfloat16, tag="fpt")
                        nc.sync.dma_start(pt[:, :FR], p0_d[:, y0 * X:y1 * X])
                        nc.vector.scalar_tensor_tensor(
                            scr[:, :FR], mb[:, :FR], 1.0, pt[:, :FR],
                            aop.mult, aop.mult, accum_out=acc[0:ZO, 8 + ci:9 + ci])
                nc.sync.dma_start(sums_d[:, :], acc[:, :])

    nc.compile()
    return nc


def _host_shard(logits, targets):
    logits = np.ascontiguousarray(np.asarray(logits, dtype=np.float32))
    targets = np.asarray(targets)
    in_maps = []
    for c in range(NCORES):
        b, zh, yh = c >> 2, (c >> 1) & 1, c & 1
        lg = logits[b]
        gt = (targets[b] == 1)
        if zh:
            lg = lg[:, ::-1]
            gt = gt[::-1]
        if yh:
            lg = lg[:, :, ::-1]
            gt = gt[:, ::-1]
        lg = np.ascontiguousarray(lg[:, :ZL, :YL]).astype(_mld.bfloat16)
        gt = np.ascontiguousarray(gt[:ZL, :YL])               # (ZL, YL, X) bool
        words = np.packbits(gt, axis=-1, bitorder="little")   # (ZL, YL, 24) u8
        words = words.view(np.uint32)                         # (ZL, YL, 6)
        gtb = np.zeros((ZL, YL, NW), dtype=np.uint32)
        gtb[:, :, 1:7] = words
        gtf = np.ascontiguousarray(gt[:ZO, :YO]).reshape(ZO, YO * X).astype(_mld.bfloat16)
        in_maps.append({"lg": lg, "gtb": gtb, "gtf": gtf})
    return in_maps


def _reduce_sums(per_core_sums):
    S = np.zeros(4, dtype=np.float64)
    for a in per_core_sums:
        a = a.astype(np.float64)
        S[0] += a[:, 0:4].sum()
        S[1] += a[:, 4:8].sum()
        S[2] += a[:, 8:12].sum()
        S[3] += a[:, 12:16].sum()
    tprec = (S[0] + 1.0) / (S[1] + 1.0)
    tsens = (S[2] + 1.0) / (S[3] + 1.0)
    cl = 2.0 * tprec * tsens / (tprec + tsens + 1e-7)
    return np.float32(1.0 - cl)


def kernel(logits, targets):
    from concourse.bass_utils import run_bass_kernel_spmd
    if "nc" not in _CACHE:
        _CACHE["nc"] = _build()
    nc = _CACHE["nc"]
    in_maps = _host_shard(logits, targets)
    res = run_bass_kernel_spmd(nc, in_maps, list(range(NCORES)), trace=False)
    return _reduce_sums([r["sums"] for r in res.results])


# revision 4
# speedup vs baseline: 2.0798x; 1.0785x over previous
"""CLDice loss Trainium2 kernel (v2).

Sharding: 8 cores = (batch b, z-half, y-half) quarters with 12-voxel halos on
interior z/y cut sides; lower/right shards are z/y-flipped by the host so the
true volume border is always at index 0.

Per core the soft-skeletonization runs entirely in SBUF:
  - pred path: bf16 [z=108 partitions, (y=108)x(x=192) free]; 3^3 min/max
    pools decompose per axis into 2 tensor_tensor ops; z +-1 shifts are
    SBUF->SBUF partition-shifted DMAs (no PE matmuls / PSUM);
    the x stage pairs taps as s'[x]=op(d[x-1],d[x+1]) then op(s',d) so only
    one op is 4B-misaligned.
  - skeleton update uses skel += delta*(1-skel) with delta = e_j - open(e_j)
    which is >= 0 exactly (opening <= id holds exactly for min/max), so no
    relu ops are needed.
  - gt path: bit-packed uint32 boolean morphology (AND/OR), same structure.
Partial sums are combined on the host into the scalar loss.
"""
import numpy as np

NCORES = 8
Z = Y = X = 192
ZO = YO = 96          # owned extent per quarter
HALO = 12             # 11 erodes + 1 dilate
ZL = YL = ZO + HALO   # local extended extent (108)
NW = 8                # words per row in packed gt (cols 1..6 data, 0/7 pads)
ND = 6                # data words per row
NIT = 11              # skeletonize iterations (incl. k=0 init)
FCH = 24              # final-pass row chunk
FY = YL * X
ONES = 0xFFFFFFFF

_CACHE = {}

import ml_dtypes as _mld

# PE shift matrices: out[p] = src[min(p+1, ZL-1)] (up) / src[max(p-1, 0)] (dn).
# The clamps implement SAME-padding at the z borders for free.
_SU = np.zeros((ZL, 128), dtype=_mld.bfloat16)
_SD = np.zeros((ZL, 128), dtype=_mld.bfloat16)
for _m in range(ZL):
    _SU[min(_m + 1, ZL - 1), _m] = 1
    _SD[max(_m - 1, 0), _m] = 1


def _build(reps=1):
    import concourse.bacc as bacc
    import concourse.bass as bass
    import concourse.mybir as mybir
    from concourse import tile
    from concourse.alu_op_type import AluOpType as aop
    from contextlib import nullcontext

    dt = mybir.dt
    AF = mybir.ActivationFunctionType
    nc = bacc.Bacc("TRN2", target_bir_lowering=False, debug=False,
                   num_devices=NCORES)

    lg_d = nc.dram_tensor("lg", [2, ZL, YL, X], dt.bfloat16, kind="ExternalInput").ap()
    gtb_d = nc.dram_tensor("gtb", [ZL, YL, NW], dt.uint32, kind="ExternalInput").ap()
    gtf_d = nc.dram_tensor("gtf", [ZO, YO * X], dt.bfloat16, kind="ExternalInput").ap()
    su_d = nc.dram_tensor("su", [ZL, 128], dt.bfloat16, kind="ExternalInput").ap()
    sd_d = nc.dram_tensor("sd", [ZL, 128], dt.bfloat16, kind="ExternalInput").ap()
    p0_d = nc.dram_tensor("p0", [ZO, YO * X], dt.bfloat16).ap()
    sums_d = nc.dram_tensor("sums", [128, 16], dt.float32, kind="ExternalOutput").ap()

    with tile.TileContext(nc) as tc:
        with tc.tile_pool(name="perm", bufs=1) as perm:
            skel = perm.tile([ZO, YO * X], dt.bfloat16)
            skg = perm.tile([ZO, YO * NW], dt.uint32)
            acc = perm.tile([128, 16], dt.float32)
            loop = tc.For_i(0, reps, 1) if reps > 1 else nullcontext()
            with loop:
                nc.vector.memset(acc[:, :], 0.0)
                with tc.tile_pool(name="mp", bufs=1) as mp, \
                     tc.tile_pool(name="pp", bufs=1,
                                  space=bass.MemorySpace.PSUM) as pp:
                    su_t = mp.tile([ZL, 128], dt.bfloat16, name="su_t")
                    sd_t = mp.tile([ZL, 128], dt.bfloat16, name="sd_t")
                    nc.sync.dma_start(su_t[:, :], su_d[:, :])
                    nc.sync.dma_start(sd_t[:, :], sd_d[:, :])
                    Ea = mp.tile([ZL, FY], dt.bfloat16, name="Ea")
                    Eb = mp.tile([ZL, FY], dt.bfloat16, name="Eb")
                    S1 = mp.tile([ZL, FY], dt.bfloat16, name="S1")
                    D1 = mp.tile([ZL, 51 * X], dt.bfloat16, name="D1")
                    DTt = mp.tile([ZO, 16 * X], dt.bfloat16, name="DTt")
                    Ga = mp.tile([ZL, YL * NW], dt.uint32, name="Ga")
                    Gb = mp.tile([ZL, YL * NW], dt.uint32, name="Gb")
                    T1 = mp.tile([ZL, YL * NW], dt.uint32, name="T1")
                    T2 = mp.tile([ZL, YL * NW], dt.uint32, name="T2")
                    T3 = mp.tile([ZL, YL * NW], dt.uint32, name="T3")

                    # ---------------- init: sigmoid(l1-l0) -> Ea --------------
                    nc.vector.memset(T1[:, :], 0)
                    nc.vector.memset(T2[:, :], 0)
                    nc.vector.memset(T3[:, :], 0)
                    nc.sync.dma_start(S1[:, :], lg_d[0, :, :, :])
                    nc.sync.dma_start(Eb[:, :], lg_d[1, :, :, :])
                    nc.sync.dma_start(Ga[:, :], gtb_d[:, :, :])
                    nc.vector.tensor_sub(S1[:, :], Eb[:, :], S1[:, :])
                    nc.scalar.activation(Ea[:, :], S1[:, :], AF.Sigmoid)
                    nc.sync.dma_start(p0_d[:, :], Ea[0:ZO, 0:YO * X])

                    # -------- z +-1 shifts via PE shift-matmuls into PSUM ----
                    # (SBUF<->SBUF partition-shift DMA only runs at ~22 GB/s
                    # here, ~360us per 4.4MB; PE + ScalarE evacuation is far
                    # off the critical path instead.)
                    def pe_shift(src, c_lo, c_hi, dstU, du_off, dstD, dd_off):
                        w = c_hi - c_lo
                        for c0 in range(0, w, 2048):
                            cw = min(2048, w - c0)
                            psU = pp.tile([128, 2048], dt.float32, tag="psU")
                            psD = pp.tile([128, 2048], dt.float32, tag="psD")
                            for j0 in range(0, cw, 512):
                                jw = min(512, cw - j0)
                                s = src[:, c_lo + c0 + j0:c_lo + c0 + j0 + jw]
                                nc.tensor.matmul(psU[:, j0:j0 + jw], su_t[:, :], s)
                                nc.tensor.matmul(psD[:, j0:j0 + jw], sd_t[:, :], s)
                            nc.scalar.copy(dstU[0:ZL, du_off + c0:du_off + c0 + cw],
                                           psU[0:ZL, 0:cw])
                            nc.scalar.copy(dstD[0:ZL, dd_off + c0:dd_off + c0 + cw],
                                           psD[0:ZL, 0:cw])

                    # ---------------- float erode: A -> B (via S1) ------------
                    def erode(A, B):
                        R = YL - 1  # 107
                        # z stage: b = min(A, A[z+1], A[z-1])
                        pe_shift(A, 0, FY, S1, 0, B, 0)
                        nc.vector.tensor_tensor(B[:, :], A[:, :], B[:, :], aop.min)
                        nc.vector.tensor_tensor(B[:, :], B[:, :], S1[:, :], aop.min)
                        # y stage: c[y]=min(b[y],b[y+1]); d[0]=c[0]; d[y]=min(c[y-1],c[y])
                        nc.vector.tensor_tensor(S1[:, 0:R * X], B[:, 0:R * X], B[:, X:YL * X], aop.min)
                        nc.scalar.copy(B[:, 0:X], S1[:, 0:X])
                        nc.vector.tensor_tensor(B[:, X:R * X], S1[:, 0:(R - 1) * X], S1[:, X:R * X], aop.min)
                        # x stage on rows 0..R-1
                        B3 = B.rearrange("p (r c) -> p r c", c=X)
                        S3 = S1.rearrange("p (r c) -> p r c", c=X)
                        nc.vector.tensor_tensor(S3[:, 0:R, 0:X - 2], B3[:, 0:R, 0:X - 2], B3[:, 0:R, 2:X], aop.min)
                        nc.vector.tensor_tensor(B3[:, 0:R, X - 1:X], B3[:, 0:R, X - 2:X - 1], B3[:, 0:R, X - 1:X], aop.min)
                        nc.vector.tensor_tensor(B3[:, 0:R, 0:1], B3[:, 0:R, 0:1], B3[:, 0:R, 1:2], aop.min)
                        nc.vector.tensor_tensor(B3[:, 0:R, 1:X - 1], S3[:, 0:R, 0:X - 2], B3[:, 0:R, 1:X - 1], aop.min)

                    # -------- float dilate of B (=e_{j+1}) + delta/skel -------
                    def dilate_delta(A, B, j):
                        ZP = ZO + 1  # 97: z rows 0..96 of the dilate
                        for y0, y1 in ((0, 48), (48, YO)):
                            w0 = max(y0 - 1, 0)
                            w1 = y1 + 1
                            L = w1 - w0           # 49 / 50
                            Lc = y1 - w0          # 48 / 49
                            LX = L * X
                            # z stage on y rows w0..w1-1
                            pe_shift(B, w0 * X, w1 * X, S1, 0, D1, 0)
                            nc.vector.tensor_tensor(S1[0:ZP, 0:LX], B[0:ZP, w0 * X:w1 * X], S1[0:ZP, 0:LX], aop.max)
                            nc.vector.tensor_tensor(D1[0:ZP, 0:LX], S1[0:ZP, 0:LX], D1[0:ZP, 0:LX], aop.max)
                            # y stage
                            nc.vector.tensor_tensor(S1[0:ZP, 0:Lc * X], D1[0:ZP, 0:Lc * X], D1[0:ZP, X:(Lc + 1) * X], aop.max)
                            if y0 == 0:
                                nc.scalar.copy(D1[0:ZP, 0:X], S1[0:ZP, 0:X])
                                nc.vector.tensor_tensor(D1[0:ZP, X:48 * X], S1[0:ZP, 0:47 * X], S1[0:ZP, X:48 * X], aop.max)
                            else:
                                nc.vector.tensor_tensor(D1[0:ZP, 0:48 * X], S1[0:ZP, 0:48 * X], S1[0:ZP, X:49 * X], aop.max)
                            # x stage rows 0..47
                            RD = 48
                            D3 = D1.rearrange("p (r c) -> p r c", c=X)
                            S3 = S1.rearrange("p (r c) -> p r c", c=X)
                            nc.vector.tensor_tensor(S3[0:ZP, 0:RD, 0:X - 2], D3[0:ZP, 0:RD, 0:X - 2], D3[0:ZP, 0:RD, 2:X], aop.max)
                            nc.vector.tensor_tensor(D3[0:ZP, 0:RD, X - 1:X], D3[0:ZP, 0:RD, X - 2:X - 1], D3[0:ZP, 0:RD, X - 1:X], aop.max)
                            nc.vector.tensor_tensor(D3[0:ZP, 0:RD, 0:1], D3[0:ZP, 0:RD, 0:1], D3[0:ZP, 0:RD, 1:2], aop.max)
                            nc.vector.tensor_tensor(D3[0:ZP, 0:RD, 1:X - 1], S3[0:ZP, 0:RD, 0:X - 2], D3[0:ZP, 0:RD, 1:X - 1], aop.max)
                            # delta/skel on 16-row chunks of the owned half
                            for q in range(3):
                                g0 = y0 + q * 16
                                l0 = q * 16
                                cs = slice(g0 * X, (g0 + 16) * X)
                                ls = slice(l0 * X, (l0 + 16) * X)
                                nc.gpsimd.tensor_sub(DTt[:, :], A[0:ZO, cs], D1[0:ZO, ls])
                                if j == 0:
                                    nc.vector.tensor_copy(skel[:, cs], DTt[:, :])
                                else:
                                    nc.scalar.activation(D1[0:ZO, ls], skel[:, cs], AF.Copy, scale=-1.0, bias=1.0)
                                    nc.gpsimd.tensor_mul(DTt[:, :], DTt[:, :], D1[0:ZO, ls])
                                    nc.gpsimd.tensor_add(skel[:, cs], skel[:, cs], DTt[:, :])

                    # ---------------- packed-gt helpers -----------------------
                    def gt_pads(G, val):
                        G3 = G.rearrange("p (r w) -> p r w", w=NW)
                        nc.vector.memset(G3[:, :, 0:1], val)
                        nc.vector.memset(G3[:, :, 7:8], val)

                    def gt_erode(G, H):
                        R = YL - 1
                        gt_pads(G, ONES)
                        nc.sync.dma_start(T1[0:ZL - 1, :], G[1:ZL, :])
                        nc.vector.tensor_tensor(T1[:, :], G[:, :], T1[:, :], aop.bitwise_and)
                        nc.sync.dma_start(H[1:ZL, :], G[0:ZL - 1, :])
                        nc.vector.memset(H[0:1, :], ONES)
                        nc.vector.tensor_tensor(H[:, :], T1[:, :], H[:, :], aop.bitwise_and)
                        nc.vector.tensor_tensor(T1[:, 0:R * NW], H[:, 0:R * NW], H[:, NW:YL * NW], aop.bitwise_and)
                        nc.vector.tensor_copy(H[:, 0:NW], T1[:, 0:NW])
                        nc.vector.tensor_tensor(H[:, NW:R * NW], T1[:, 0:(R - 1) * NW], T1[:, NW:R * NW], aop.bitwise_and)
                        # x bits: out = (H & t_minus) & t_plus
                        H3 = H.rearrange("p (r w) -> p r w", w=NW)
                        t13 = T1.rearrange("p (r w) -> p r w", w=NW)
                        t23 = T2.rearrange("p (r w) -> p r w", w=NW)
                        t33 = T3.rearrange("p (r w) -> p r w", w=NW)
                        v = slice(0, R)
                        nc.vector.tensor_single_scalar(t13[:, v, 1:7], H3[:, v, 1:7], 1, aop.logical_shift_left)
                        nc.vector.tensor_single_scalar(t23[:, v, 1:7], H3[:, v, 0:6], 31, aop.logical_shift_right)
                        nc.vector.tensor_tensor(t13[:, v, 1:7], t13[:, v, 1:7], t23[:, v, 1:7], aop.bitwise_or)
                        nc.vector.tensor_tensor(t13[:, v, 1:7], t13[:, v, 1:7], H3[:, v, 1:7], aop.bitwise_and)
                        nc.vector.tensor_single_scalar(t23[:, v, 1:7], H3[:, v, 1:7], 1, aop.logical_shift_right)
                        nc.vector.tensor_single_scalar(t33[:, v, 1:7], H3[:, v, 2:8], 31, aop.logical_shift_left)
                        nc.vector.tensor_tensor(t23[:, v, 1:7], t23[:, v, 1:7], t33[:, v, 1:7], aop.bitwise_or)
                        nc.vector.tensor_tensor(H3[:, v, 1:7], t13[:, v, 1:7], t23[:, v, 1:7], aop.bitwise_and)

                    def gt_dilate_skg(G, H, j):
                        R = YL - 1
                        gt_pads(H, 0)
                        nc.sync.dma_start(T1[0:ZL - 1, :], H[1:ZL, :])
                        nc.vector.tensor_tensor(T1[:, :], H[:, :], T1[:, :], aop.bitwise_or)
                        nc.sync.dma_start(T2[1:ZL, :], H[0:ZL - 1, :])
                        nc.vector.memset(T2[0:1, :], 0)
                        nc.vector.tensor_tensor(T2[:, :], T1[:, :], T2[:, :], aop.bitwise_or)
                        nc.vector.tensor_tensor(T1[:, 0:R * NW], T2[:, 0:R * NW], T2[:, NW:YL * NW], aop.bitwise_or)
                        nc.vector.tensor_copy(T2[:, 0:NW], T1[:, 0:NW])
                        nc.vector.tensor_tensor(T2[:, NW:R * NW], T1[:, 0:(R - 1) * NW], T1[:, NW:R * NW], aop.bitwise_or)
                        # x bits (OR distributes): GD = d | d<<1 | prev>>31 | d>>1 | next<<31
                        t13 = T1.rearrange("p (r w) -> p r w", w=NW)
                        t23 = T2.rearrange("p (r w) -> p r w", w=NW)
                        t33 = T3.rearrange("p (r w) -> p r w", w=NW)
                        v = slice(0, R)
                        nc.vector.tensor_single_scalar(t13[:, v, 1:7], t23[:, v, 1:7], 1, aop.logical_shift_left)
                        nc.vector.tensor_tensor(t13[:, v, 1:7], t13[:, v, 1:7], t23[:, v, 1:7], aop.bitwise_or)
                        nc.vector.tensor_single_scalar(t33[:, v, 1:7], t23[:, v, 0:6], 31, aop.logical_shift_right)
                        nc.vector.tensor_tensor(t13[:, v, 1:7], t13[:, v, 1:7], t33[:, v, 1:7], aop.bitwise_or)
                        nc.vector.tensor_single_scalar(t33[:, v, 1:7], t23[:, v, 1:7], 1, aop.logical_shift_right)
                        nc.vector.tensor_tensor(t13[:, v, 1:7], t13[:, v, 1:7], t33[:, v, 1:7], aop.bitwise_or)
                        nc.vector.tensor_single_scalar(t33[:, v, 1:7], t23[:, v, 2:8], 31, aop.logical_shift_left)
                        nc.vector.tensor_tensor(t13[:, v, 1:7], t13[:, v, 1:7], t33[:, v, 1:7], aop.bitwise_or)
                        # gnt = G & ~GD on owned; fold into skg
                        G3 = G.rearrange("p (r w) -> p r w", w=NW)
                        skg3 = skg.rearrange("p (r w) -> p r w", w=NW)
                        nc.vector.tensor_single_scalar(t13[0:ZO, 0:YO, 1:7], t13[0:ZO, 0:YO, 1:7], ONES, aop.bitwise_xor)
                        nc.vector.tensor_tensor(t13[0:ZO, 0:YO, 1:7], G3[0:ZO, 0:YO, 1:7], t13[0:ZO, 0:YO, 1:7], aop.bitwise_and)
                        if j == 0:
                            nc.vector.tensor_copy(skg3[:, :, 1:7], t13[0:ZO, 0:YO, 1:7])
                        else:
                            nc.vector.tensor_tensor(skg3[:, :, 1:7], skg3[:, :, 1:7], t13[0:ZO, 0:YO, 1:7], aop.bitwise_or)

                    # ---------------- main iterations -------------------------
                    cur, nxt = Ea, Eb
                    gcur, gnxt = Ga, Gb
                    for j in range(NIT):
                        erode(cur, nxt)
                        dilate_delta(cur, nxt, j)
                        gt_erode(gcur, gnxt)
                        gt_dilate_skg(gcur, gnxt, j)
                        cur, nxt = nxt, cur
                        gcur, gnxt = gnxt, gcur

                # ---------------- final: partial sums ----------------------
                with tc.tile_pool(name="fin", bufs=1) as fp:
                    for ci in range(4):
                        y0, y1 = ci * FCH, (ci + 1) * FCH
                        rr = y1 - y0
                        FR = rr * X
                        # S1 = sum(skel_pred * gt): gt comes in as bf16
                        gf = fp.tile([ZO, FCH * X], dt.bfloat16, tag="fgf")
                        nc.sync.dma_start(gf[:, :FR], gtf_d[:, y0 * X:y1 * X])
                        scr = fp.tile([ZO, FCH * X], dt.bfloat16, tag="fsc")
                        nc.vector.scalar_tensor_tensor(
                            scr[:, :FR], skel[:, y0 * X:y1 * X], 1.0, gf[:, :FR],
                            aop.mult, aop.mult, accum_out=acc[0:ZO, ci:ci + 1])
                        # S2 = sum(skel_pred)
                        nc.vector.tensor_scalar(
                            scr[:, :FR], skel[:, y0 * X:y1 * X], 0.0, 0.0,
                            aop.add, aop.add, accum_out=acc[0:ZO, 4 + ci:5 + ci])
                        # unpack skel_gt
                        mi = fp.tile([ZO, FCH * X], dt.uint32, tag="fmi")
                        mi4 = mi.rearrange("p (r w b) -> p r w b", w=ND, b=32)
                        sg4 = skg.rearrange("p (r w) -> p r w", w=NW)
                        for b in range(32):
                            nc.vector.tensor_scalar(
                                mi4[:, 0:rr, :, b], sg4[:, y0:y1, 1:7], b, 1,
                                aop.logical_shift_right, aop.bitwise_and)
                        mb = fp.tile([ZO, FCH * X], dt.bfloat16, tag="fmb")
                        nc.scalar.copy(mb[:, :FR], mi[:, :FR])
                        # S4 = sum(skel_gt)
                        nc.vector.tensor_scalar(
                            scr[:, :FR], mb[:, :FR], 0.0, 0.0,
                            aop.add, aop.add, accum_out=acc[0:ZO, 12 + ci:13 + ci])
                        # S3 = sum(skel_gt * pred)
                        pt = fp.tile([ZO, FCH * X], dt.bfloat16, tag="fpt")
                        nc.sync.dma_start(pt[:, :FR], p0_d[:, y0 * X:y1 * X])
                        nc.vector.scalar_tensor_tensor(
                            scr[:, :FR], mb[:, :FR], 1.0, pt[:, :FR],
                            aop.mult, aop.mult, accum_out=acc[0:ZO, 8 + ci:9 + ci])
                nc.sync.dma_start(sums_d[:, :], acc[:, :])

    nc.compile()
    return nc


def _host_shard(logits, targets):
    logits = np.ascontiguousarray(np.asarray(logits, dtype=np.float32))
    targets = np.asarray(targets)
    in_maps = []
    for c in range(NCORES):
        b, zh, yh = c >> 2, (c >> 1) & 1, c & 1
        lg = logits[b]
        gt = (targets[b] == 1)
        if zh:
            lg = lg[:, ::-1]
            gt = gt[::-1]
        if yh:
            lg = lg[:, :, ::-1]
            gt = gt[:, ::-1]
        lg = np.ascontiguousarray(lg[:, :ZL, :YL]).astype(_mld.bfloat16)
        gt = np.ascontiguousarray(gt[:ZL, :YL])               # (ZL, YL, X) bool
        words = np.packbits(gt, axis=-1, bitorder="little")   # (ZL, YL, 24) u8
        words = words.view(np.uint32)                         # (ZL, YL, 6)
        gtb = np.zeros((ZL, YL, NW), dtype=np.uint32)
        gtb[:, :, 1:7] = words
        gtf = np.ascontiguousarray(gt[:ZO, :YO]).reshape(ZO, YO * X).astype(_mld.bfloat16)
        in_maps.append({"lg": lg, "gtb": gtb, "gtf": gtf, "su": _SU, "sd": _SD})
    return in_maps


def _reduce_sums(per_core_sums):
    S = np.zeros(4, dtype=np.float64)
    for a in per_core_sums:
        a = a.astype(np.float64)
        S[0] += a[:, 0:4].sum()
        S[1] += a[:, 4:8].sum()
        S[2] += a[:, 8:12].sum()
        S[3] += a[:, 12:16].sum()
    tprec = (S[0] + 1.0) / (S[1] + 1.0)
    tsens = (S[2] + 1.0) / (S[3] + 1.0)
    cl = 2.0 * tprec * tsens / (tprec + tsens + 1e-7)
    return np.float32(1.0 - cl)


def kernel(logits, targets):
    from concourse.bass_utils import run_bass_kernel_spmd
    if "nc" not in _CACHE:
        _CACHE["nc"] = _build()
    nc = _CACHE["nc"]
    in_maps = _host_shard(logits, targets)
    res = run_bass_kernel_spmd(nc, in_maps, list(range(NCORES)), trace=False)
    return _reduce_sums([r["sums"] for r in res.results])
